# revision 4
# baseline (speedup 1.0000x reference)
"""Trainium2 Bass kernel for nn_Diffuser_78331613544465 (v2b).

Math (per graph b of B=8, N=1024):
    A   = adj (mask all-ones in graded setup; general mask handled host-side)
    P   = A / max(rowsum(A), 1)
    out[i,j,:] = relu([I, P, P2, P4][i,j,:] @ w1 + b1) @ w2 + b2

Device strategy: one graph per NeuronCore, all work in the transposed
domain Q = P^T (Q = A * invdeg-col-scale since A is symmetric):
  * squares use NO PE transposes: lhsT tiles for X@X come from XBAR
    DMA-transposed copies (P = Q^T, P2 = Q2^T) built by dma_start_transpose.
  * edge MLP: layer 1 runs as 4 concurrent 32x128 PE tiles (tile_position),
    K=32 = 3 matrix channels x 8 j-rows + 8 identity-matrix rows (the
    identity channel realizes the self-adjacency stack); layer 2 runs as
    4 concurrent 64x32 tiles with block-diagonal w2.
  * relu splits each round between scalar and vector engines (gpsimd has
    no PSUM access); final po -> fp16 -> XBAR transpose -> one big DMA per
    (q,K,half) into a [1024,1024,8] fp16 output; host only casts to f32.

kernel(**inputs) takes FULL inputs, shards over 8 cores, returns FULL output.
"""

import os
import numpy as np

B, N, P = 8, 1024, 128
HID, HEADS = 16, 8
NT = N // P          # 8 row-tiles
IC = 512             # matmul free-dim chunk
NIC = N // IC        # 2
NJJ = 32             # j-blocks per row-group (each block = 8 j rows)
NK = 8               # K-groups of 4 j-blocks

_CACHE = {}
LAST_RESULTS = None


def _emit(nc, tc, ctx):
    import concourse.bass as bass
    from concourse import mybir

    f32 = mybir.dt.float32
    f16 = mybir.dt.float16
    relu = mybir.ActivationFunctionType.Relu
    ADD = mybir.AluOpType.add
    MAX = mybir.AluOpType.max

    adj = nc.declare_dram_parameter("adj", [N, N], f32, isOutput=False)
    eyeD = nc.declare_dram_parameter("eye16", [N, N], f16, isOutput=False)
    w1big_d = nc.declare_dram_parameter("w1big", [P, P], f16, isOutput=False)
    w2big_d = nc.declare_dram_parameter("w2big", [P, 32], f16, isOutput=False)
    b1rep_d = nc.declare_dram_parameter("b1rep", [P, 1], f32, isOutput=False)
    idn32_d = nc.declare_dram_parameter("idn32", [P, P], f32, isOutput=False)
    out = nc.declare_dram_parameter("out16", [N, N, HEADS], f16, isOutput=True)

    from contextlib import ExitStack

    # ---- persistent SBUF ----------------------------------------------------
    small = ctx.enter_context(tc.tile_pool(name="small", bufs=1))
    ilpool = ctx.enter_context(tc.tile_pool(name="ilpool", bufs=1))

    w1s = small.tile([P, P], f16, tag="w1s")
    nc.scalar.dma_start(w1s[:], w1big_d[:])
    w2s = small.tile([P, 32], f16, tag="w2s")
    nc.scalar.dma_start(w2s[:], w2big_d[:])
    b1s = small.tile([P, 1], f32, tag="b1s")
    nc.scalar.dma_start(b1s[:], b1rep_d[:])
    idn32 = small.tile([P, P], f32, tag="idn32")
    nc.scalar.dma_start(idn32[:], idn32_d[:])
    ones1 = small.tile([1, P], f32, tag="ones1")
    nc.vector.memset(ones1[:], 1.0)
    invrep = small.tile([P, N], f32, tag="invrep")
    invcol = small.tile([P, NT], f32, tag="invcol")
    invrow = small.tile([1, N], f32, tag="invrow")

    # interleaved MLP rhs: row 32q+8s+kk = M_s[256q + 8jj + kk, :] at free jj*1024+c
    ilbig = ilpool.tile([P, NJJ * N], f16, tag="ilbig")

    # DRAM spills (interleaved-load sources for the MLP)
    dram = ctx.enter_context(tc.tile_pool(name="dram", bufs=1, space="DRAM"))
    Qd = dram.tile([N, N], f16, tag="Qd")
    Q2d = dram.tile([N, N], f16, tag="Q2d")
    Q4d = dram.tile([N, N], f16, tag="Q4d")

    # ---- squares-phase SBUF scope ------------------------------------------
    sqs = ExitStack()
    sqpool = sqs.enter_context(tc.tile_pool(name="sqpool", bufs=1))
    Qf = sqpool.tile([P, NT * N], f16, tag="Qf")      # Q[128t+p, c] at [p, 1024t+c]
    Pbig = sqpool.tile([P, NT * N], f16, tag="Pbig")  # P[128t+p, 128u+l] at [p, 1024u+128t+l]
    Q2f = sqpool.tile([P, NT * N], f16, tag="Q2f")
    P2big = sqpool.tile([P, NT * N], f16, tag="P2big")
    Q4f = sqpool.tile([P, NT * N], f16, tag="Q4f")

    # ---- phase 1: load A, deg -> inv, Q = A * invrep ------------------------
    ph1 = ExitStack()
    apool = ph1.enter_context(tc.tile_pool(name="apool", bufs=1))
    pt_ps = ph1.enter_context(tc.tile_pool(name="pt_ps", bufs=2, space="PSUM"))
    mm_ps = ph1.enter_context(tc.tile_pool(name="mm_ps", bufs=2, space="PSUM"))

    Af = apool.tile([P, NT * N], f32, tag="Af")
    for t in range(NT):
        nc.sync.dma_start(Af[:, N * t:N * (t + 1)], adj[P * t:P * (t + 1), :])
        deg = small.tile([P, 1], f32, tag=f"deg{t}", name=f"deg{t}")
        nc.vector.tensor_reduce(
            deg[:], Af[:, N * t:N * (t + 1)],
            axis=mybir.AxisListType.X, op=ADD,
        )
        degc = small.tile([P, 1], f32, tag=f"degc{t}", name=f"degc{t}")
        nc.vector.tensor_scalar_max(degc[:], deg[:], 1.0)
        nc.vector.reciprocal(invcol[:, t:t + 1], degc[:])

    for t in range(NT):
        ptp = pt_ps.tile([P, P], f32, tag="pt")
        nc.tensor.transpose(ptp[0:1, :], invcol[:, t:t + 1], idn32[:])
        nc.scalar.copy(invrow[0:1, P * t:P * (t + 1)], ptp[0:1, :])
    for half in range(2):
        pb = mm_ps.tile([P, IC], f32, tag="mm")
        for k in range(4):
            c = 4 * half + k
            nc.tensor.matmul(
                pb[:, P * k:P * (k + 1)], ones1[:], invrow[0:1, P * c:P * (c + 1)],
                start=True, stop=True,
            )
        nc.scalar.copy(invrep[:, IC * half:IC * (half + 1)], pb[:])

    for t in range(NT):
        nc.vector.tensor_mul(
            Qf[:, N * t:N * (t + 1)], Af[:, N * t:N * (t + 1)], invrep[:]
        )
    ph1.close()

    # spill Q (one DMA) and build P = Q^T via XBAR strips
    nc.sync.dma_start(
        Qd[:].rearrange("(t p) c -> p t c", p=P),
        Qf[:].rearrange("p (t c) -> p t c", c=N),
    )
    for u in range(NT):
        nc.sync.dma_start_transpose(
            Pbig[:, N * u:N * (u + 1)].rearrange("p (e l) -> p e l", l=P),
            Qf[:, N * u:N * (u + 1)],
        )

    # ---- squares: X2[al,be-chunk] = sum_g lhsT(g,al).T @ X[g, be] -----------
    sq_ps_stack = ExitStack()
    sq_ps = sq_ps_stack.enter_context(tc.tile_pool(name="sq_ps", bufs=3, space="PSUM"))

    def square(lhsTbig, src, dst, dstd):
        cp = 0
        for al in range(NT):
            for be in range(NIC):
                mm = sq_ps.tile([P, IC], f32, tag="mm")
                for g in range(NT):
                    nc.tensor.matmul(
                        mm[:],
                        lhsTbig[:, N * al + P * g:N * al + P * (g + 1)],
                        src[:, N * g + IC * be:N * g + IC * (be + 1)],
                        start=(g == 0), stop=(g == NT - 1),
                    )
                dslice = dst[:, N * al + IC * be:N * al + IC * (be + 1)]
                if cp % 2 == 0:
                    nc.scalar.copy(dslice, mm[:])
                else:
                    nc.vector.tensor_copy(dslice, mm[:])
                cp += 1
        nc.sync.dma_start(
            dstd[:].rearrange("(t p) c -> p t c", p=P),
            dst[:].rearrange("p (t c) -> p t c", c=N),
        )

    square(Pbig, Qf, Q2f, Q2d)
    for u in range(NT):
        nc.sync.dma_start_transpose(
            P2big[:, N * u:N * (u + 1)].rearrange("p (e l) -> p e l", l=P),
            Q2f[:, N * u:N * (u + 1)],
        )
    square(P2big, Q2f, Q4f, Q4d)
    sq_ps_stack.close()

    # ---- MLP input loads (interleaved) --------------------------------------
    for q in range(4):
        for s, srcd in enumerate((Qd, Q2d, Q4d, eyeD)):
            eng = nc.gpsimd if s % 2 == 0 else nc.scalar
            eng.dma_start(
                ilbig[32 * q + 8 * s:32 * q + 8 * s + 8, :],
                srcd[256 * q:256 * (q + 1), :].rearrange(
                    "(jj kk) c -> kk jj c", kk=8
                ),
            )
    sqs.close()

    # ---- MLP ----------------------------------------------------------------
    mlp = ExitStack()
    rtpool = mlp.enter_context(tc.tile_pool(name="rtpool", bufs=6))
    popool = mlp.enter_context(tc.tile_pool(name="popool", bufs=10))
    otpool = mlp.enter_context(tc.tile_pool(name="otpool", bufs=4))
    h_ps = mlp.enter_context(tc.tile_pool(name="h_ps", bufs=1, space="PSUM"))
    po_ps = mlp.enter_context(tc.tile_pool(name="po_ps", bufs=4, space="PSUM"))

    for K in range(NK):
        po16 = {}
        for q in range(4):
            for half in range(2):
                po16[(q, half)] = popool.tile(
                    [P, 2 * IC], f16, tag="po16", name="po16"
                )
        for ic in range(NIC):
            rts = {}
            for jj4 in range(4):
                jj = 4 * K + jj4
                hquad = h_ps.tile([P, 4 * IC], f32, tag="hq", name="hq")
                for q in range(4):
                    nc.tensor.matmul(
                        hquad[:, IC * q:IC * (q + 1)],
                        w1s[32 * q:32 * (q + 1), :],
                        ilbig[32 * q:32 * (q + 1), N * jj + IC * ic:N * jj + IC * (ic + 1)],
                        start=True, stop=True, tile_position=(32 * q, 0),
                    )
                rtq = rtpool.tile([P, 4 * IC], f16, tag="rt", name="rt")
                nc.scalar.activation(
                    rtq[:, 0:2 * IC], hquad[:, 0:2 * IC], relu,
                    bias=b1s[:], scale=1.0,
                )
                nc.vector.tensor_scalar(
                    rtq[:, 2 * IC:4 * IC], hquad[:, 2 * IC:4 * IC],
                    b1s[:], 0.0, op0=ADD, op1=MAX,
                )
                rts[jj4] = rtq
            pcnt = 0
            for q in range(4):
                for half in range(2):
                    po = po_ps.tile([P, IC], f32, tag="po")
                    for c2 in range(2):
                        rt = rts[2 * half + c2]
                        for rg in range(2):
                            c = 2 * c2 + rg
                            nc.tensor.matmul(
                                po[32 * c:32 * (c + 1), :],
                                w2s[64 * rg:64 * (rg + 1), :],
                                rt[64 * rg:64 * (rg + 1), IC * q:IC * (q + 1)],
                                start=True, stop=True,
                                tile_position=(64 * rg, 32 * c),
                            )
                    dst16 = po16[(q, half)][:, IC * ic:IC * (ic + 1)]
                    if pcnt % 2 == 0:
                        nc.scalar.copy(dst16, po[:])
                    else:
                        nc.vector.tensor_copy(dst16, po[:])
                    pcnt += 1
        for q in range(4):
            for half in range(2):
                ot = otpool.tile([P, 8, P], f16, tag="ot", name="ot")
                nc.sync.dma_start_transpose(ot[:], po16[(q, half)][:])
                jb = 256 * q + 32 * K + 16 * half
                nc.gpsimd.dma_start(
                    out[:, jb:jb + 16, :].rearrange(
                        "(ie p) jl o -> p ie (jl o)", p=P
                    ),
                    ot[:],
                )
    mlp.close()


def _build():
    key = "v2b"
    if key in _CACHE:
        return _CACHE[key]
    from contextlib import ExitStack
    import concourse.tile as tile
    from concourse import bacc

    nc = bacc.Bacc()
    with tile.TileContext(nc) as tc:
        with ExitStack() as ctx:
            _emit(nc, tc, ctx)
    nc.compile()
    _CACHE[key] = nc
    return nc


def _install_ntff_shim():
    """Provide antenv.axon_hooks + ctypes NTFF hook so
    run_bass_kernel_spmd(trace=True) can profile."""
    import sys
    import types

    if "antenv.axon_hooks" in sys.modules:
        return
    mod = types.ModuleType("antenv.axon_hooks")
    mod._hook = None
    mod.set_axon_ntff_profile_hook = lambda h: setattr(mod, "_hook", h)
    mod.get_axon_ntff_profile_hook = lambda: mod._hook
    sys.modules["antenv.axon_hooks"] = mod
    try:
        from trn_agent_boot.trn_boot import _ntff_profile_via_ctypes

        mod._hook = _ntff_profile_via_ctypes("/opt/axon/libaxon_pjrt.so")
    except Exception as e:  # degrade to no-trace
        print(f"ntff shim install failed: {e}")


_EYE16 = None


def kernel(adj, mask, w1, b1, w2, b2):
    from concourse.bass_utils import run_bass_kernel_spmd

    global LAST_RESULTS, _EYE16
    adj = np.ascontiguousarray(np.asarray(adj, dtype=np.float32))
    mask = np.asarray(mask)
    w1 = np.asarray(w1, dtype=np.float32)
    b1 = np.asarray(b1, dtype=np.float32)
    w2 = np.asarray(w2, dtype=np.float32)
    b2 = np.asarray(b2, dtype=np.float32)
    assert adj.shape == (B, N, N), adj.shape

    m = mask.astype(np.float32)
    general_mask = not np.all(m == 1.0)
    if general_mask:
        pair = m[:, :, None] * m[:, None, :]
        adj = np.ascontiguousarray(adj * pair)

    trace = bool(int(os.environ.get("KERNEL_TRACE", "0")))
    if trace:
        _install_ntff_shim()
    nc = _build()

    if _EYE16 is None:
        _EYE16 = np.eye(N, dtype=np.float16)

    # w1big[32q + 8s + kk, 16jl + h] = delta(kk,jl) * w1[perm(s), h]
    perm = [1, 2, 3, 0]  # channels Q, Q2, Q4, eye -> stacks P1, P2, P4, self
    w1big = np.zeros((32, P), np.float32)
    for s in range(4):
        for k in range(8):
            w1big[8 * s + k, HID * k:HID * (k + 1)] = w1[perm[s]]
    w1big = np.tile(w1big, (4, 1)).astype(np.float16)
    # w2big[64r + 16jl + h, 8jl' + o] = delta(jl,jl') * w2[h, o]
    w2q = np.zeros((64, 32), np.float32)
    for jl in range(4):
        w2q[HID * jl:HID * (jl + 1), HEADS * jl:HEADS * (jl + 1)] = w2
    w2big = np.tile(w2q, (2, 1)).astype(np.float16)

    shared = {
        "eye16": _EYE16,
        "w1big": np.ascontiguousarray(w1big),
        "w2big": np.ascontiguousarray(w2big),
        "b1rep": np.ascontiguousarray(np.tile(b1, 8).astype(np.float32)[:, None]),
        "idn32": np.eye(P, dtype=np.float32),
    }
    in_maps = [{"adj": adj[c], **shared} for c in range(B)]
    res = run_bass_kernel_spmd(nc, in_maps, list(range(B)), trace=trace)
    LAST_RESULTS = res

    outp = np.stack(
        [res.results[c]["out16"].astype(np.float32) for c in range(B)], axis=0
    )
    if np.any(b2 != 0.0):
        outp = outp + b2
    if general_mask:
        outp = outp * pair[..., None]
    return np.ascontiguousarray(outp.astype(np.float32))


# revision 8
# speedup vs baseline: 1.2573x; 1.2573x over previous
"""Trainium2 Bass kernel for nn_Diffuser_78331613544465 (v3).

Math (per graph b of B=8, N=1024):
    A   = adj (mask all-ones in graded setup; general mask handled host-side)
    P   = A / max(rowsum(A), 1)
    out[i,j,:] = relu([I, P, P2, P4][i,j,:] @ w1 + b1) @ w2 + b2

Device strategy: one graph per NeuronCore, all work in the transposed
domain Q = P^T (Q = A * invdeg-col-scale since A is symmetric):
  * squares use NO PE transposes: lhsT tiles for X@X come from XBAR
    DMA-transposed copies (P = Q^T, P2 = Q2^T) built by dma_start_transpose,
    emitted per-strip so they overlap the squares.
  * edge MLP: layer 1 runs as row-tiled K=32 PE tiles (tile_position),
    K=32 = 3 matrix channels x 8 j-rows + 8 identity-matrix rows (the
    identity channel realizes the self-adjacency stack); layer 2 runs as
    4 concurrent 64x32 tiles with block-diagonal w2. Rounds use 2-bank
    PSUM h-pairs at pipeline depth 3; relu splits each round across the
    scalar AND vector engines (gpsimd has no PSUM access). L2 emission is
    software-pipelined one block behind L1 so the PE never waits on relu.
  * po -> fp16 staging per (q,K) -> one XBAR transpose -> two big DMAs
    into a [1024,1024,8] fp16 output; host only casts to f32.

kernel(**inputs) takes FULL inputs, shards over 8 cores, returns FULL output.
"""

import os
import numpy as np

B, N, P = 8, 1024, 128
HID, HEADS = 16, 8
NT = N // P          # 8 row-tiles
IC = 512             # matmul free-dim chunk
NIC = N // IC        # 2
NJJ = 32             # j-blocks per row-group (each block = 8 j rows)
NK = 8               # K-groups of 4 j-blocks

_CACHE = {}
LAST_RESULTS = None


def _emit(nc, tc, ctx):
    import concourse.bass as bass
    from concourse import mybir

    f32 = mybir.dt.float32
    f16 = mybir.dt.float16
    relu = mybir.ActivationFunctionType.Relu
    ADD = mybir.AluOpType.add
    MAX = mybir.AluOpType.max

    adj = nc.declare_dram_parameter("adj", [N, N], f32, isOutput=False)
    eyeD = nc.declare_dram_parameter("eye16", [N, N], f16, isOutput=False)
    w1big_d = nc.declare_dram_parameter("w1big", [P, P], f16, isOutput=False)
    w2big_d = nc.declare_dram_parameter("w2big", [P, 32], f16, isOutput=False)
    b1rep_d = nc.declare_dram_parameter("b1rep", [P, 1], f32, isOutput=False)
    idn32_d = nc.declare_dram_parameter("idn32", [P, P], f32, isOutput=False)
    out = nc.declare_dram_parameter("out16", [N, N, HEADS], f16, isOutput=True)

    from contextlib import ExitStack

    # ---- persistent SBUF ----------------------------------------------------
    small = ctx.enter_context(tc.tile_pool(name="small", bufs=1))
    ilpool = ctx.enter_context(tc.tile_pool(name="ilpool", bufs=1))

    w1s = small.tile([P, P], f16, tag="w1s")
    nc.scalar.dma_start(w1s[:], w1big_d[:])
    w2s = small.tile([P, 32], f16, tag="w2s")
    nc.scalar.dma_start(w2s[:], w2big_d[:])
    b1s = small.tile([P, 1], f32, tag="b1s")
    nc.scalar.dma_start(b1s[:], b1rep_d[:])
    idn32 = small.tile([P, P], f32, tag="idn32")
    nc.scalar.dma_start(idn32[:], idn32_d[:])
    ones1 = small.tile([1, P], f32, tag="ones1")
    nc.vector.memset(ones1[:], 1.0)
    invrep = small.tile([P, N], f32, tag="invrep")
    invcol = small.tile([P, NT], f32, tag="invcol")
    invT = small.tile([1, N], f32, tag="invT")

    # interleaved MLP rhs: row 32q+8s+kk = M_s[256q + 8jj + kk, :] at free jj*1024+c
    ilbig = ilpool.tile([P, NJJ * N], f16, tag="ilbig")

    # DRAM spills (interleaved-load sources for the MLP)
    dram = ctx.enter_context(tc.tile_pool(name="dram", bufs=1, space="DRAM"))
    Qd = dram.tile([N, N], f16, tag="Qd")
    Q2d = dram.tile([N, N], f16, tag="Q2d")
    Q4d = dram.tile([N, N], f16, tag="Q4d")

    def ile_load(eng, s, srcd, q):
        eng.dma_start(
            ilbig[32 * q + 8 * s:32 * q + 8 * s + 8, :],
            srcd[256 * q:256 * (q + 1), :].rearrange("(jj kk) c -> kk jj c", kk=8),
        )

    # identity channel: independent of everything, load first
    for q in range(4):
        ile_load(nc.gpsimd, 3, eyeD, q)

    # ---- squares-phase SBUF scope ------------------------------------------
    sqs = ExitStack()
    sqpool = sqs.enter_context(tc.tile_pool(name="sqpool", bufs=1))
    Qf = sqpool.tile([P, NT * N], f16, tag="Qf")      # Q[128t+p, c] at [p, 1024t+c]
    Pbig = sqpool.tile([P, NT * N], f16, tag="Pbig")  # P[128t+p, 128u+l] at [p, 1024u+128t+l]
    Q2f = sqpool.tile([P, NT * N], f16, tag="Q2f")
    P2big = sqpool.tile([P, NT * N], f16, tag="P2big")
    Q4f = sqpool.tile([P, NT * N], f16, tag="Q4f")

    # ---- phase 1: load A, deg -> inv, Q = A * invrep, P = Q^T ---------------
    ph1 = ExitStack()
    apool = ph1.enter_context(tc.tile_pool(name="apool", bufs=1))
    pt_ps = ph1.enter_context(tc.tile_pool(name="pt_ps", bufs=1, space="PSUM"))
    mm_ps = ph1.enter_context(tc.tile_pool(name="mm_ps", bufs=2, space="PSUM"))

    Af = apool.tile([P, NT * N], f32, tag="Af")
    for t in range(NT):
        nc.sync.dma_start(Af[:, N * t:N * (t + 1)], adj[P * t:P * (t + 1), :])
        deg = small.tile([P, 1], f32, tag=f"deg{t}", name=f"deg{t}")
        nc.vector.tensor_reduce(
            deg[:], Af[:, N * t:N * (t + 1)],
            axis=mybir.AxisListType.X, op=ADD,
        )
        degc = small.tile([P, 1], f32, tag=f"degc{t}", name=f"degc{t}")
        nc.vector.tensor_scalar_max(degc[:], deg[:], 1.0)
        nc.vector.reciprocal(invcol[:, t:t + 1], degc[:])

    for t in range(NT):
        ptp = pt_ps.tile([P, P], f32, tag="pt")
        nc.tensor.transpose(ptp[0:1, :], invcol[:, t:t + 1], idn32[:])
        nc.scalar.copy(invT[0:1, P * t:P * (t + 1)], ptp[0:1, :])
    for half in range(2):
        pb = mm_ps.tile([P, IC], f32, tag="mm")
        for k in range(4):
            c = 4 * half + k
            nc.tensor.matmul(
                pb[:, P * k:P * (k + 1)], ones1[:], invT[0:1, P * c:P * (c + 1)],
                start=True, stop=True,
            )
        nc.scalar.copy(invrep[:, IC * half:IC * (half + 1)], pb[:])

    for t in range(NT):
        nc.vector.tensor_mul(
            Qf[:, N * t:N * (t + 1)], Af[:, N * t:N * (t + 1)], invrep[:]
        )
        nc.sync.dma_start_transpose(
            Pbig[:, N * t:N * (t + 1)].rearrange("p (e l) -> p e l", l=P),
            Qf[:, N * t:N * (t + 1)],
        )
    ph1.close()

    # spill Q, then its interleaved loads
    nc.sync.dma_start(
        Qd[:].rearrange("(t p) c -> p t c", p=P),
        Qf[:].rearrange("p (t c) -> p t c", c=N),
    )
    for q in range(4):
        ile_load(nc.gpsimd, 0, Qd, q)

    # ---- squares: X2[al,be-chunk] = sum_g lhsT(g,al).T @ X[g, be] -----------
    sq_ps_stack = ExitStack()
    sq_ps = sq_ps_stack.enter_context(tc.tile_pool(name="sq_ps", bufs=3, space="PSUM"))

    def square(lhsTbig, src, dst, post_al):
        cp = 0
        for al in range(NT):
            for be in range(NIC):
                mm = sq_ps.tile([P, IC], f32, tag="mm")
                for g in range(NT):
                    nc.tensor.matmul(
                        mm[:],
                        lhsTbig[:, N * al + P * g:N * al + P * (g + 1)],
                        src[:, N * g + IC * be:N * g + IC * (be + 1)],
                        start=(g == 0), stop=(g == NT - 1),
                    )
                dslice = dst[:, N * al + IC * be:N * al + IC * (be + 1)]
                if cp % 2 == 0:
                    nc.scalar.copy(dslice, mm[:])
                else:
                    nc.vector.tensor_copy(dslice, mm[:])
                cp += 1
            post_al(al)

    def post_al_q2(al):
        # P2 col-block al only needs Q2 strip al
        nc.sync.dma_start_transpose(
            P2big[:, N * al:N * (al + 1)].rearrange("p (e l) -> p e l", l=P),
            Q2f[:, N * al:N * (al + 1)],
        )

    square(Pbig, Qf, Q2f, post_al_q2)
    nc.sync.dma_start(
        Q2d[:].rearrange("(t p) c -> p t c", p=P),
        Q2f[:].rearrange("p (t c) -> p t c", c=N),
    )
    for q in range(4):
        ile_load(nc.scalar, 1, Q2d, q)

    def post_al_q4(al):
        if al % 2 == 1:
            q = al // 2
            nc.sync.dma_start(
                Q4d[256 * q:256 * (q + 1), :].rearrange("(t p) c -> p t c", p=P),
                Q4f[:, N * (al - 1):N * (al + 1)].rearrange("p (t c) -> p t c", c=N),
            )
            ile_load(nc.gpsimd, 2, Q4d, q)

    square(P2big, Q2f, Q4f, post_al_q4)
    sq_ps_stack.close()
    sqs.close()

    # ---- MLP ----------------------------------------------------------------
    mlp = ExitStack()
    rtpool = mlp.enter_context(tc.tile_pool(name="rtpool", bufs=20))
    popool = mlp.enter_context(tc.tile_pool(name="popool", bufs=8))
    otpool = mlp.enter_context(tc.tile_pool(name="otpool", bufs=4))
    h_ps = mlp.enter_context(tc.tile_pool(name="h_ps", bufs=3, space="PSUM"))
    po_ps = mlp.enter_context(tc.tile_pool(name="po_ps", bufs=2, space="PSUM"))

    po16 = {}

    def l1_block(K, ic):
        rts = {}
        for jj4 in range(4):
            jj = 4 * K + jj4
            for qp in range(2):
                hpair = h_ps.tile([P, 2 * IC], f32, tag="hp", name="hp")
                for ql in range(2):
                    q = 2 * qp + ql
                    nc.tensor.matmul(
                        hpair[:, IC * ql:IC * (ql + 1)],
                        w1s[32 * q:32 * (q + 1), :],
                        ilbig[32 * q:32 * (q + 1), N * jj + IC * ic:N * jj + IC * (ic + 1)],
                        start=True, stop=True, tile_position=(32 * q, 0),
                    )
                rt = rtpool.tile([P, 2 * IC], f16, tag="rt", name="rt")
                nc.scalar.activation(
                    rt[:, 0:IC], hpair[:, 0:IC], relu, bias=b1s[:], scale=1.0,
                )
                nc.vector.tensor_scalar(
                    rt[:, IC:2 * IC], hpair[:, IC:2 * IC], b1s[:], 0.0,
                    op0=ADD, op1=MAX,
                )
                rts[(jj4, qp)] = rt
        return rts

    pcnt = 0

    def l2_block(K, ic, rts):
        nonlocal pcnt
        for q in range(4):
            if (q, K) not in po16:
                po16[(q, K)] = popool.tile([P, 4 * IC], f16, tag="po16", name="po16")
            for half in range(2):
                po = po_ps.tile([P, IC], f32, tag="po")
                for c2 in range(2):
                    rt = rts[(2 * half + c2, q // 2)]
                    for rg in range(2):
                        c = 2 * c2 + rg
                        nc.tensor.matmul(
                            po[32 * c:32 * (c + 1), :],
                            w2s[64 * rg:64 * (rg + 1), :],
                            rt[64 * rg:64 * (rg + 1), IC * (q % 2):IC * (q % 2 + 1)],
                            start=True, stop=True,
                            tile_position=(64 * rg, 32 * c),
                        )
                dst16 = po16[(q, K)][:, 2 * IC * half + IC * ic:2 * IC * half + IC * (ic + 1)]
                if pcnt % 2 == 0:
                    nc.scalar.copy(dst16, po[:])
                else:
                    nc.vector.tensor_copy(dst16, po[:])
                pcnt += 1
        if ic == 1:
            for q in range(4):
                ot = otpool.tile([P, 16, P], f16, tag="ot", name="ot")
                nc.sync.dma_start_transpose(ot[:], po16[(q, K)][:])
                for half in range(2):
                    jb = 256 * q + 32 * K + 16 * half
                    nc.gpsimd.dma_start(
                        out[:, jb:jb + 16, :].rearrange(
                            "(ie p) jl o -> p ie (jl o)", p=P
                        ),
                        ot[:, 8 * half:8 * (half + 1), :],
                    )

    blocks = [(K, ic) for K in range(NK) for ic in range(NIC)]
    prev = None
    for blk in blocks:
        rts = l1_block(*blk)
        if prev is not None:
            l2_block(prev[0][0], prev[0][1], prev[1])
        prev = (blk, rts)
    l2_block(prev[0][0], prev[0][1], prev[1])
    mlp.close()


def _build():
    key = "v3"
    if key in _CACHE:
        return _CACHE[key]
    from contextlib import ExitStack
    import concourse.tile as tile
    from concourse import bacc

    nc = bacc.Bacc()
    with tile.TileContext(nc) as tc:
        with ExitStack() as ctx:
            _emit(nc, tc, ctx)
    nc.compile()
    _CACHE[key] = nc
    return nc


def _install_ntff_shim():
    """Provide antenv.axon_hooks + ctypes NTFF hook so
    run_bass_kernel_spmd(trace=True) can profile."""
    import sys
    import types

    if "antenv.axon_hooks" in sys.modules:
        return
    mod = types.ModuleType("antenv.axon_hooks")
    mod._hook = None
    mod.set_axon_ntff_profile_hook = lambda h: setattr(mod, "_hook", h)
    mod.get_axon_ntff_profile_hook = lambda: mod._hook
    sys.modules["antenv.axon_hooks"] = mod
    try:
        from trn_agent_boot.trn_boot import _ntff_profile_via_ctypes

        mod._hook = _ntff_profile_via_ctypes("/opt/axon/libaxon_pjrt.so")
    except Exception as e:  # degrade to no-trace
        print(f"ntff shim install failed: {e}")


_EYE16 = None


def kernel(adj, mask, w1, b1, w2, b2):
    from concourse.bass_utils import run_bass_kernel_spmd

    global LAST_RESULTS, _EYE16
    adj = np.ascontiguousarray(np.asarray(adj, dtype=np.float32))
    mask = np.asarray(mask)
    w1 = np.asarray(w1, dtype=np.float32)
    b1 = np.asarray(b1, dtype=np.float32)
    w2 = np.asarray(w2, dtype=np.float32)
    b2 = np.asarray(b2, dtype=np.float32)
    assert adj.shape == (B, N, N), adj.shape

    m = mask.astype(np.float32)
    general_mask = not np.all(m == 1.0)
    if general_mask:
        pair = m[:, :, None] * m[:, None, :]
        adj = np.ascontiguousarray(adj * pair)

    trace = bool(int(os.environ.get("KERNEL_TRACE", "0")))
    if trace:
        _install_ntff_shim()
    nc = _build()

    if _EYE16 is None:
        _EYE16 = np.eye(N, dtype=np.float16)

    # w1big[32q + 8s + kk, 16jl + h] = delta(kk,jl) * w1[perm(s), h]
    perm = [1, 2, 3, 0]  # channels Q, Q2, Q4, eye -> stacks P1, P2, P4, self
    w1big = np.zeros((32, P), np.float32)
    for s in range(4):
        for k in range(8):
            w1big[8 * s + k, HID * k:HID * (k + 1)] = w1[perm[s]]
    w1big = np.tile(w1big, (4, 1)).astype(np.float16)
    # w2big[64r + 16jl + h, 8jl' + o] = delta(jl,jl') * w2[h, o]
    w2q = np.zeros((64, 32), np.float32)
    for jl in range(4):
        w2q[HID * jl:HID * (jl + 1), HEADS * jl:HEADS * (jl + 1)] = w2
    w2big = np.tile(w2q, (2, 1)).astype(np.float16)

    shared = {
        "eye16": _EYE16,
        "w1big": np.ascontiguousarray(w1big),
        "w2big": np.ascontiguousarray(w2big),
        "b1rep": np.ascontiguousarray(np.tile(b1, 8).astype(np.float32)[:, None]),
        "idn32": np.eye(P, dtype=np.float32),
    }
    in_maps = [{"adj": adj[c], **shared} for c in range(B)]
    res = run_bass_kernel_spmd(nc, in_maps, list(range(B)), trace=trace)
    LAST_RESULTS = res

    outp = np.stack(
        [res.results[c]["out16"].astype(np.float32) for c in range(B)], axis=0
    )
    if np.any(b2 != 0.0):
        outp = outp + b2
    if general_mask:
        outp = outp * pair[..., None]
    return np.ascontiguousarray(outp.astype(np.float32))


# revision 9
# speedup vs baseline: 1.5572x; 1.2385x over previous
"""Trainium2 Bass kernel for nn_Diffuser_78331613544465 (v3).

Math (per graph b of B=8, N=1024):
    A   = adj (mask all-ones in graded setup; general mask handled host-side)
    P   = A / max(rowsum(A), 1)
    out[i,j,:] = relu([I, P, P2, P4][i,j,:] @ w1 + b1) @ w2 + b2

Device strategy: one graph per NeuronCore, all work in the transposed
domain Q = P^T (Q = A * invdeg-col-scale since A is symmetric):
  * squares use NO PE transposes: lhsT tiles for X@X come from XBAR
    DMA-transposed copies (P = Q^T, P2 = Q2^T) built by dma_start_transpose,
    emitted per-strip so they overlap the squares.
  * edge MLP: layer 1 runs as row-tiled K=32 PE tiles (tile_position),
    K=32 = 3 matrix channels x 8 j-rows + 8 identity-matrix rows (the
    identity channel realizes the self-adjacency stack); layer 2 runs as
    4 concurrent 64x32 tiles with block-diagonal w2. Rounds use 2-bank
    PSUM h-pairs at pipeline depth 3; relu splits each round across the
    scalar AND vector engines (gpsimd has no PSUM access). L2 emission is
    software-pipelined one block behind L1 so the PE never waits on relu.
  * po -> fp16 staging per (q,K) -> one XBAR transpose -> two big DMAs
    into a [1024,1024,8] fp16 output; host only casts to f32.

kernel(**inputs) takes FULL inputs, shards over 8 cores, returns FULL output.
"""

import os
import numpy as np

B, N, P = 8, 1024, 128
HID, HEADS = 16, 8
NT = N // P          # 8 row-tiles
IC = 512             # matmul free-dim chunk
NIC = N // IC        # 2
NJJ = 32             # j-blocks per row-group (each block = 8 j rows)
NK = 8               # K-groups of 4 j-blocks

_CACHE = {}
LAST_RESULTS = None


def _emit(nc, tc, ctx):
    import concourse.bass as bass
    from concourse import mybir

    f32 = mybir.dt.float32
    f16 = mybir.dt.float16
    relu = mybir.ActivationFunctionType.Relu
    ADD = mybir.AluOpType.add
    MAX = mybir.AluOpType.max

    adj = nc.declare_dram_parameter("adj", [N, N], f32, isOutput=False)
    eyeD = nc.declare_dram_parameter("eye16", [N, N], f16, isOutput=False)
    w1big_d = nc.declare_dram_parameter("w1big", [P, P], f16, isOutput=False)
    w2big_d = nc.declare_dram_parameter("w2big", [P, 32], f16, isOutput=False)
    b1rep_d = nc.declare_dram_parameter("b1rep", [P, 1], f32, isOutput=False)
    idn32_d = nc.declare_dram_parameter("idn32", [P, P], f32, isOutput=False)
    out = nc.declare_dram_parameter(
        "out_raw", [4, NK, P, 2, NIC, IC], f16, isOutput=True
    )

    from contextlib import ExitStack

    # ---- persistent SBUF ----------------------------------------------------
    small = ctx.enter_context(tc.tile_pool(name="small", bufs=1))
    ilpool = ctx.enter_context(tc.tile_pool(name="ilpool", bufs=1))

    w1s = small.tile([P, P], f16, tag="w1s")
    nc.scalar.dma_start(w1s[:], w1big_d[:])
    w2s = small.tile([P, 32], f16, tag="w2s")
    nc.scalar.dma_start(w2s[:], w2big_d[:])
    b1s = small.tile([P, 1], f32, tag="b1s")
    nc.scalar.dma_start(b1s[:], b1rep_d[:])
    idn32 = small.tile([P, P], f32, tag="idn32")
    nc.scalar.dma_start(idn32[:], idn32_d[:])
    ones1 = small.tile([1, P], f32, tag="ones1")
    nc.vector.memset(ones1[:], 1.0)
    invrep = small.tile([P, N], f32, tag="invrep")
    invcol = small.tile([P, NT], f32, tag="invcol")
    invT = small.tile([1, N], f32, tag="invT")

    # interleaved MLP rhs: row 32q+8s+kk = M_s[256q + 8jj + kk, :] at free jj*1024+c
    ilbig = ilpool.tile([P, NJJ * N], f16, tag="ilbig")

    # DRAM spills (interleaved-load sources for the MLP)
    dram = ctx.enter_context(tc.tile_pool(name="dram", bufs=1, space="DRAM"))
    Qd = dram.tile([N, N], f16, tag="Qd")
    Q2d = dram.tile([N, N], f16, tag="Q2d")
    Q4d = dram.tile([N, N], f16, tag="Q4d")

    def ile_load(eng, s, srcd, q):
        eng.dma_start(
            ilbig[32 * q + 8 * s:32 * q + 8 * s + 8, :],
            srcd[256 * q:256 * (q + 1), :].rearrange("(jj kk) c -> kk jj c", kk=8),
        )

    # identity channel: independent of everything, load first
    for q in range(4):
        ile_load(nc.gpsimd, 3, eyeD, q)

    # ---- squares-phase SBUF scope ------------------------------------------
    sqs = ExitStack()
    sqpool = sqs.enter_context(tc.tile_pool(name="sqpool", bufs=1))
    Qf = sqpool.tile([P, NT * N], f16, tag="Qf")      # Q[128t+p, c] at [p, 1024t+c]
    Pbig = sqpool.tile([P, NT * N], f16, tag="Pbig")  # P[128t+p, 128u+l] at [p, 1024u+128t+l]
    Q2f = sqpool.tile([P, NT * N], f16, tag="Q2f")
    P2big = sqpool.tile([P, NT * N], f16, tag="P2big")
    Q4f = sqpool.tile([P, NT * N], f16, tag="Q4f")

    # ---- phase 1: load A, deg -> inv, Q = A * invrep, P = Q^T ---------------
    ph1 = ExitStack()
    apool = ph1.enter_context(tc.tile_pool(name="apool", bufs=1))
    pt_ps = ph1.enter_context(tc.tile_pool(name="pt_ps", bufs=1, space="PSUM"))
    mm_ps = ph1.enter_context(tc.tile_pool(name="mm_ps", bufs=2, space="PSUM"))

    Af = apool.tile([P, NT * N], f32, tag="Af")
    for t in range(NT):
        nc.sync.dma_start(Af[:, N * t:N * (t + 1)], adj[P * t:P * (t + 1), :])
        deg = small.tile([P, 1], f32, tag=f"deg{t}", name=f"deg{t}")
        nc.vector.tensor_reduce(
            deg[:], Af[:, N * t:N * (t + 1)],
            axis=mybir.AxisListType.X, op=ADD,
        )
        degc = small.tile([P, 1], f32, tag=f"degc{t}", name=f"degc{t}")
        nc.vector.tensor_scalar_max(degc[:], deg[:], 1.0)
        nc.vector.reciprocal(invcol[:, t:t + 1], degc[:])

    for t in range(NT):
        ptp = pt_ps.tile([P, P], f32, tag="pt")
        nc.tensor.transpose(ptp[0:1, :], invcol[:, t:t + 1], idn32[:])
        nc.scalar.copy(invT[0:1, P * t:P * (t + 1)], ptp[0:1, :])
    for half in range(2):
        pb = mm_ps.tile([P, IC], f32, tag="mm")
        for k in range(4):
            c = 4 * half + k
            nc.tensor.matmul(
                pb[:, P * k:P * (k + 1)], ones1[:], invT[0:1, P * c:P * (c + 1)],
                start=True, stop=True,
            )
        nc.scalar.copy(invrep[:, IC * half:IC * (half + 1)], pb[:])

    for t in range(NT):
        nc.vector.tensor_mul(
            Qf[:, N * t:N * (t + 1)], Af[:, N * t:N * (t + 1)], invrep[:]
        )
        nc.sync.dma_start_transpose(
            Pbig[:, N * t:N * (t + 1)].rearrange("p (e l) -> p e l", l=P),
            Qf[:, N * t:N * (t + 1)],
        )
    ph1.close()

    # spill Q, then its interleaved loads
    nc.sync.dma_start(
        Qd[:].rearrange("(t p) c -> p t c", p=P),
        Qf[:].rearrange("p (t c) -> p t c", c=N),
    )
    for q in range(4):
        ile_load(nc.gpsimd, 0, Qd, q)

    # ---- squares: X2[al,be-chunk] = sum_g lhsT(g,al).T @ X[g, be] -----------
    sq_ps_stack = ExitStack()
    sq_ps = sq_ps_stack.enter_context(tc.tile_pool(name="sq_ps", bufs=6, space="PSUM"))

    def square(lhsTbig, src, dst, post_al):
        cp = 0
        for al in range(NT):
            for be in range(NIC):
                mm = sq_ps.tile([P, IC], f32, tag="mm")
                for g in range(NT):
                    nc.tensor.matmul(
                        mm[:],
                        lhsTbig[:, N * al + P * g:N * al + P * (g + 1)],
                        src[:, N * g + IC * be:N * g + IC * (be + 1)],
                        start=(g == 0), stop=(g == NT - 1),
                    )
                dslice = dst[:, N * al + IC * be:N * al + IC * (be + 1)]
                if cp % 2 == 0:
                    nc.scalar.copy(dslice, mm[:])
                else:
                    nc.vector.tensor_copy(dslice, mm[:])
                cp += 1
            post_al(al)

    def post_al_q2(al):
        # P2 col-block al only needs Q2 strip al
        nc.sync.dma_start_transpose(
            P2big[:, N * al:N * (al + 1)].rearrange("p (e l) -> p e l", l=P),
            Q2f[:, N * al:N * (al + 1)],
        )

    square(Pbig, Qf, Q2f, post_al_q2)
    nc.sync.dma_start(
        Q2d[:].rearrange("(t p) c -> p t c", p=P),
        Q2f[:].rearrange("p (t c) -> p t c", c=N),
    )
    for q in range(4):
        ile_load(nc.gpsimd, 1, Q2d, q)

    def post_al_q4(al):
        if al % 2 == 1:
            q = al // 2
            nc.sync.dma_start(
                Q4d[256 * q:256 * (q + 1), :].rearrange("(t p) c -> p t c", p=P),
                Q4f[:, N * (al - 1):N * (al + 1)].rearrange("p (t c) -> p t c", c=N),
            )
            ile_load(nc.gpsimd, 2, Q4d, q)

    square(P2big, Q2f, Q4f, post_al_q4)
    sq_ps_stack.close()
    sqs.close()

    # ---- MLP ----------------------------------------------------------------
    mlp = ExitStack()
    rtpool = mlp.enter_context(tc.tile_pool(name="rtpool", bufs=20))
    popool = mlp.enter_context(tc.tile_pool(name="popool", bufs=8))
    h_ps = mlp.enter_context(tc.tile_pool(name="h_ps", bufs=3, space="PSUM"))
    po_ps = mlp.enter_context(tc.tile_pool(name="po_ps", bufs=2, space="PSUM"))

    po16 = {}

    def l1_block(K, ic):
        rts = {}
        for jj4 in range(4):
            jj = 4 * K + jj4
            for qp in range(2):
                hpair = h_ps.tile([P, 2 * IC], f32, tag="hp", name="hp")
                for ql in range(2):
                    q = 2 * qp + ql
                    nc.tensor.matmul(
                        hpair[:, IC * ql:IC * (ql + 1)],
                        w1s[32 * q:32 * (q + 1), :],
                        ilbig[32 * q:32 * (q + 1), N * jj + IC * ic:N * jj + IC * (ic + 1)],
                        start=True, stop=True, tile_position=(32 * q, 0),
                    )
                rt = rtpool.tile([P, 2 * IC], f16, tag="rt", name="rt")
                if (jj4 + qp) % 2 == 0:
                    nc.scalar.activation(
                        rt[:], hpair[:], relu, bias=b1s[:], scale=1.0,
                    )
                else:
                    nc.vector.tensor_scalar(
                        rt[:], hpair[:], b1s[:], 0.0, op0=ADD, op1=MAX,
                    )
                rts[(jj4, qp)] = rt
        return rts

    pcnt = 0

    def l2_block(K, ic, rts):
        nonlocal pcnt
        for q in range(4):
            if (q, K) not in po16:
                po16[(q, K)] = popool.tile([P, 4 * IC], f16, tag="po16", name="po16")
            for half in range(2):
                po = po_ps.tile([P, IC], f32, tag="po")
                for c2 in range(2):
                    rt = rts[(2 * half + c2, q // 2)]
                    for rg in range(2):
                        c = 2 * c2 + rg
                        nc.tensor.matmul(
                            po[32 * c:32 * (c + 1), :],
                            w2s[64 * rg:64 * (rg + 1), :],
                            rt[64 * rg:64 * (rg + 1), IC * (q % 2):IC * (q % 2 + 1)],
                            start=True, stop=True,
                            tile_position=(64 * rg, 32 * c),
                        )
                dst16 = po16[(q, K)][:, 2 * IC * half + IC * ic:2 * IC * half + IC * (ic + 1)]
                if pcnt % 2 == 0:
                    nc.scalar.copy(dst16, po[:])
                else:
                    nc.vector.tensor_copy(dst16, po[:])
                pcnt += 1
        if ic == 1:
            for q in range(4):
                nc.gpsimd.dma_start(
                    out[q, K].rearrange("jo half ic i -> jo (half ic i)"),
                    po16[(q, K)][:],
                )

    blocks = [(K, ic) for K in range(NK) for ic in range(NIC)]
    prev = None
    for blk in blocks:
        rts = l1_block(*blk)
        if prev is not None:
            l2_block(prev[0][0], prev[0][1], prev[1])
        prev = (blk, rts)
    l2_block(prev[0][0], prev[0][1], prev[1])
    mlp.close()


def _build():
    key = "v4"
    if key in _CACHE:
        return _CACHE[key]
    from contextlib import ExitStack
    import concourse.tile as tile
    from concourse import bacc

    nc = bacc.Bacc()
    with tile.TileContext(nc) as tc:
        with ExitStack() as ctx:
            _emit(nc, tc, ctx)
    nc.compile()
    _CACHE[key] = nc
    return nc


def _install_ntff_shim():
    """Provide antenv.axon_hooks + ctypes NTFF hook so
    run_bass_kernel_spmd(trace=True) can profile."""
    import sys
    import types

    if "antenv.axon_hooks" in sys.modules:
        return
    mod = types.ModuleType("antenv.axon_hooks")
    mod._hook = None
    mod.set_axon_ntff_profile_hook = lambda h: setattr(mod, "_hook", h)
    mod.get_axon_ntff_profile_hook = lambda: mod._hook
    sys.modules["antenv.axon_hooks"] = mod
    try:
        from trn_agent_boot.trn_boot import _ntff_profile_via_ctypes

        mod._hook = _ntff_profile_via_ctypes("/opt/axon/libaxon_pjrt.so")
    except Exception as e:  # degrade to no-trace
        print(f"ntff shim install failed: {e}")


_EYE16 = None


def kernel(adj, mask, w1, b1, w2, b2):
    from concourse.bass_utils import run_bass_kernel_spmd

    global LAST_RESULTS, _EYE16
    adj = np.ascontiguousarray(np.asarray(adj, dtype=np.float32))
    mask = np.asarray(mask)
    w1 = np.asarray(w1, dtype=np.float32)
    b1 = np.asarray(b1, dtype=np.float32)
    w2 = np.asarray(w2, dtype=np.float32)
    b2 = np.asarray(b2, dtype=np.float32)
    assert adj.shape == (B, N, N), adj.shape

    m = mask.astype(np.float32)
    general_mask = not np.all(m == 1.0)
    if general_mask:
        pair = m[:, :, None] * m[:, None, :]
        adj = np.ascontiguousarray(adj * pair)

    trace = bool(int(os.environ.get("KERNEL_TRACE", "0")))
    if trace:
        _install_ntff_shim()
    nc = _build()

    if _EYE16 is None:
        _EYE16 = np.eye(N, dtype=np.float16)

    # w1big[32q + 8s + kk, 16jl + h] = delta(kk,jl) * w1[perm(s), h]
    perm = [1, 2, 3, 0]  # channels Q, Q2, Q4, eye -> stacks P1, P2, P4, self
    w1big = np.zeros((32, P), np.float32)
    for s in range(4):
        for k in range(8):
            w1big[8 * s + k, HID * k:HID * (k + 1)] = w1[perm[s]]
    w1big = np.tile(w1big, (4, 1)).astype(np.float16)
    # w2big[64r + 16jl + h, 8jl' + o] = delta(jl,jl') * w2[h, o]
    w2q = np.zeros((64, 32), np.float32)
    for jl in range(4):
        w2q[HID * jl:HID * (jl + 1), HEADS * jl:HEADS * (jl + 1)] = w2
    w2big = np.tile(w2q, (2, 1)).astype(np.float16)

    shared = {
        "eye16": _EYE16,
        "w1big": np.ascontiguousarray(w1big),
        "w2big": np.ascontiguousarray(w2big),
        "b1rep": np.ascontiguousarray(np.tile(b1, 8).astype(np.float32)[:, None]),
        "idn32": np.eye(P, dtype=np.float32),
    }
    in_maps = [{"adj": adj[c], **shared} for c in range(B)]
    res = run_bass_kernel_spmd(nc, in_maps, list(range(B)), trace=trace)
    LAST_RESULTS = res

    # out_raw[q, K, (16jl x 8o), half, ic, i'] -> out[i, j, o]
    # j = 256q + 32K + 16half + jl ; i = 512ic + i'
    outs = []
    for c in range(B):
        v = res.results[c]["out_raw"].reshape(4, NK, 16, HEADS, 2, NIC, IC)
        o = np.transpose(v, (5, 6, 0, 1, 4, 2, 3)).reshape(N, N, HEADS)
        outs.append(o.astype(np.float32))
    outp = np.stack(outs, axis=0)
    if np.any(b2 != 0.0):
        outp = outp + b2
    if general_mask:
        outp = outp * pair[..., None]
    return np.ascontiguousarray(outp.astype(np.float32))


# revision 10
# speedup vs baseline: 1.5947x; 1.0241x over previous
"""Trainium2 Bass kernel for nn_Diffuser_78331613544465 (v3).

Math (per graph b of B=8, N=1024):
    A   = adj (mask all-ones in graded setup; general mask handled host-side)
    P   = A / max(rowsum(A), 1)
    out[i,j,:] = relu([I, P, P2, P4][i,j,:] @ w1 + b1) @ w2 + b2

Device strategy: one graph per NeuronCore, all work in the transposed
domain Q = P^T (Q = A * invdeg-col-scale since A is symmetric):
  * squares use NO PE transposes: lhsT tiles for X@X come from XBAR
    DMA-transposed copies (P = Q^T, P2 = Q2^T) built by dma_start_transpose,
    emitted per-strip so they overlap the squares.
  * edge MLP: layer 1 runs as row-tiled K=32 PE tiles (tile_position),
    K=32 = 3 matrix channels x 8 j-rows + 8 identity-matrix rows (the
    identity channel realizes the self-adjacency stack); layer 2 runs as
    4 concurrent 64x32 tiles with block-diagonal w2. Rounds use 2-bank
    PSUM h-pairs at pipeline depth 3; relu splits each round across the
    scalar AND vector engines (gpsimd has no PSUM access). L2 emission is
    software-pipelined one block behind L1 so the PE never waits on relu.
  * po -> fp16 staging per (q,K) -> one XBAR transpose -> two big DMAs
    into a [1024,1024,8] fp16 output; host only casts to f32.

kernel(**inputs) takes FULL inputs, shards over 8 cores, returns FULL output.
"""

import os
import numpy as np

B, N, P = 8, 1024, 128
HID, HEADS = 16, 8
NT = N // P          # 8 row-tiles
IC = 512             # matmul free-dim chunk
NIC = N // IC        # 2
NJJ = 32             # j-blocks per row-group (each block = 8 j rows)
NK = 8               # K-groups of 4 j-blocks

_CACHE = {}
LAST_RESULTS = None


def _emit(nc, tc, ctx):
    import concourse.bass as bass
    from concourse import mybir

    f32 = mybir.dt.float32
    f16 = mybir.dt.float16
    relu = mybir.ActivationFunctionType.Relu
    ADD = mybir.AluOpType.add
    MAX = mybir.AluOpType.max

    adj = nc.declare_dram_parameter("adj", [N, N], f32, isOutput=False)
    eyeD = nc.declare_dram_parameter("eye16", [N, N], f16, isOutput=False)
    w1big_d = nc.declare_dram_parameter("w1big", [P, P], f16, isOutput=False)
    w2big_d = nc.declare_dram_parameter("w2big", [P, 32], f16, isOutput=False)
    b1rep_d = nc.declare_dram_parameter("b1rep", [P, 1], f32, isOutput=False)
    idn32_d = nc.declare_dram_parameter("idn32", [P, P], f32, isOutput=False)
    out = nc.declare_dram_parameter(
        "out_raw", [4, NK, P, 2, NIC, IC], f16, isOutput=True
    )

    from contextlib import ExitStack

    # ---- persistent SBUF ----------------------------------------------------
    small = ctx.enter_context(tc.tile_pool(name="small", bufs=1))
    ilpool = ctx.enter_context(tc.tile_pool(name="ilpool", bufs=1))

    w1s = small.tile([P, P], f16, tag="w1s")
    nc.scalar.dma_start(w1s[:], w1big_d[:])
    w2s = small.tile([P, 32], f16, tag="w2s")
    nc.scalar.dma_start(w2s[:], w2big_d[:])
    b1s = small.tile([P, 1], f32, tag="b1s")
    nc.scalar.dma_start(b1s[:], b1rep_d[:])
    idn32 = small.tile([P, P], f32, tag="idn32")
    nc.scalar.dma_start(idn32[:], idn32_d[:])
    ones1 = small.tile([1, P], f32, tag="ones1")
    nc.vector.memset(ones1[:], 1.0)
    invrep = small.tile([P, N], f32, tag="invrep")
    invcol = small.tile([P, NT], f32, tag="invcol")
    invT = small.tile([1, N], f32, tag="invT")

    # interleaved MLP rhs: row 32q+8s+kk = M_s[256q + 8jj + kk, :] at free jj*1024+c
    ilbig = ilpool.tile([P, NJJ * N], f16, tag="ilbig")

    # DRAM spills (interleaved-load sources for the MLP)
    dram = ctx.enter_context(tc.tile_pool(name="dram", bufs=1, space="DRAM"))
    Qd = dram.tile([N, N], f16, tag="Qd")
    Q2d = dram.tile([N, N], f16, tag="Q2d")
    Q4d = dram.tile([N, N], f16, tag="Q4d")

    def ile_load(eng, s, srcd, q):
        eng.dma_start(
            ilbig[32 * q + 8 * s:32 * q + 8 * s + 8, :],
            srcd[256 * q:256 * (q + 1), :].rearrange("(jj kk) c -> kk jj c", kk=8),
        )

    # identity channel: independent of everything, load first
    for q in range(4):
        ile_load(nc.gpsimd, 3, eyeD, q)

    # ---- squares-phase SBUF scope ------------------------------------------
    sqs = ExitStack()
    sqpool = sqs.enter_context(tc.tile_pool(name="sqpool", bufs=1))
    Qf = sqpool.tile([P, NT * N], f16, tag="Qf")      # Q[128t+p, c] at [p, 1024t+c]
    Pbig = sqpool.tile([P, NT * N], f16, tag="Pbig")  # P[128t+p, 128u+l] at [p, 1024u+128t+l]
    Q2f = sqpool.tile([P, NT * N], f16, tag="Q2f")
    P2big = sqpool.tile([P, NT * N], f16, tag="P2big")
    Q4f = sqpool.tile([P, NT * N], f16, tag="Q4f")

    # ---- phase 1: load A, deg -> inv, Q = A * invrep, P = Q^T ---------------
    ph1 = ExitStack()
    apool = ph1.enter_context(tc.tile_pool(name="apool", bufs=1))
    pt_ps = ph1.enter_context(tc.tile_pool(name="pt_ps", bufs=1, space="PSUM"))
    mm_ps = ph1.enter_context(tc.tile_pool(name="mm_ps", bufs=2, space="PSUM"))

    Af = apool.tile([P, NT * N], f32, tag="Af")
    for t in range(NT):
        eng = nc.sync if t % 2 == 0 else nc.scalar
        eng.dma_start(Af[:, N * t:N * (t + 1)], adj[P * t:P * (t + 1), :])
        deg = small.tile([P, 1], f32, tag=f"deg{t}", name=f"deg{t}")
        nc.vector.tensor_reduce(
            deg[:], Af[:, N * t:N * (t + 1)],
            axis=mybir.AxisListType.X, op=ADD,
        )
        degc = small.tile([P, 1], f32, tag=f"degc{t}", name=f"degc{t}")
        nc.vector.tensor_scalar_max(degc[:], deg[:], 1.0)
        nc.vector.reciprocal(invcol[:, t:t + 1], degc[:])

    for t in range(NT):
        ptp = pt_ps.tile([P, P], f32, tag="pt")
        nc.tensor.transpose(ptp[0:1, :], invcol[:, t:t + 1], idn32[:])
        nc.scalar.copy(invT[0:1, P * t:P * (t + 1)], ptp[0:1, :])
    for half in range(2):
        pb = mm_ps.tile([P, IC], f32, tag="mm")
        for k in range(4):
            c = 4 * half + k
            nc.tensor.matmul(
                pb[:, P * k:P * (k + 1)], ones1[:], invT[0:1, P * c:P * (c + 1)],
                start=True, stop=True,
            )
        nc.scalar.copy(invrep[:, IC * half:IC * (half + 1)], pb[:])

    for t in range(NT):
        nc.vector.tensor_mul(
            Qf[:, N * t:N * (t + 1)], Af[:, N * t:N * (t + 1)], invrep[:]
        )
        nc.sync.dma_start_transpose(
            Pbig[:, N * t:N * (t + 1)].rearrange("p (e l) -> p e l", l=P),
            Qf[:, N * t:N * (t + 1)],
        )
    ph1.close()

    # spill Q, then its interleaved loads
    nc.scalar.dma_start(
        Qd[:].rearrange("(t p) c -> p t c", p=P),
        Qf[:].rearrange("p (t c) -> p t c", c=N),
    )
    for q in range(4):
        ile_load(nc.gpsimd, 0, Qd, q)

    # ---- squares: X2[al,be-chunk] = sum_g lhsT(g,al).T @ X[g, be] -----------
    sq_ps_stack = ExitStack()
    sq_ps = sq_ps_stack.enter_context(tc.tile_pool(name="sq_ps", bufs=6, space="PSUM"))

    def square(lhsTbig, src, dst, post_al):
        cp = 0
        for al in range(NT):
            for be in range(NIC):
                mm = sq_ps.tile([P, IC], f32, tag="mm")
                for g in range(NT):
                    nc.tensor.matmul(
                        mm[:],
                        lhsTbig[:, N * al + P * g:N * al + P * (g + 1)],
                        src[:, N * g + IC * be:N * g + IC * (be + 1)],
                        start=(g == 0), stop=(g == NT - 1),
                    )
                dslice = dst[:, N * al + IC * be:N * al + IC * (be + 1)]
                if cp % 2 == 0:
                    nc.scalar.copy(dslice, mm[:])
                else:
                    nc.vector.tensor_copy(dslice, mm[:])
                cp += 1
            post_al(al)

    def post_al_q2(al):
        # P2 col-block al only needs Q2 strip al
        nc.sync.dma_start_transpose(
            P2big[:, N * al:N * (al + 1)].rearrange("p (e l) -> p e l", l=P),
            Q2f[:, N * al:N * (al + 1)],
        )

    square(Pbig, Qf, Q2f, post_al_q2)
    nc.scalar.dma_start(
        Q2d[:].rearrange("(t p) c -> p t c", p=P),
        Q2f[:].rearrange("p (t c) -> p t c", c=N),
    )
    for q in range(4):
        ile_load(nc.gpsimd, 1, Q2d, q)

    def post_al_q4(al):
        if al % 2 == 1:
            q = al // 2
            nc.scalar.dma_start(
                Q4d[256 * q:256 * (q + 1), :].rearrange("(t p) c -> p t c", p=P),
                Q4f[:, N * (al - 1):N * (al + 1)].rearrange("p (t c) -> p t c", c=N),
            )
            ile_load(nc.gpsimd, 2, Q4d, q)

    square(P2big, Q2f, Q4f, post_al_q4)
    sq_ps_stack.close()
    sqs.close()

    # ---- MLP ----------------------------------------------------------------
    mlp = ExitStack()
    rtpool = mlp.enter_context(tc.tile_pool(name="rtpool", bufs=20))
    popool = mlp.enter_context(tc.tile_pool(name="popool", bufs=8))
    h_ps = mlp.enter_context(tc.tile_pool(name="h_ps", bufs=3, space="PSUM"))
    po_ps = mlp.enter_context(tc.tile_pool(name="po_ps", bufs=2, space="PSUM"))

    po16 = {}

    def l1_block(K, ic):
        rts = {}
        for jj4 in range(4):
            jj = 4 * K + jj4
            for qp in range(2):
                hpair = h_ps.tile([P, 2 * IC], f32, tag="hp", name="hp")
                for ql in range(2):
                    q = 2 * qp + ql
                    nc.tensor.matmul(
                        hpair[:, IC * ql:IC * (ql + 1)],
                        w1s[32 * q:32 * (q + 1), :],
                        ilbig[32 * q:32 * (q + 1), N * jj + IC * ic:N * jj + IC * (ic + 1)],
                        start=True, stop=True, tile_position=(32 * q, 0),
                    )
                rt = rtpool.tile([P, 2 * IC], f16, tag="rt", name="rt")
                if (jj4 + qp) % 2 == 0:
                    nc.scalar.activation(
                        rt[:], hpair[:], relu, bias=b1s[:], scale=1.0,
                    )
                else:
                    nc.vector.tensor_scalar(
                        rt[:], hpair[:], b1s[:], 0.0, op0=ADD, op1=MAX,
                    )
                rts[(jj4, qp)] = rt
        return rts

    pcnt = 0

    def l2_block(K, ic, rts):
        nonlocal pcnt
        for q in range(4):
            if (q, K) not in po16:
                po16[(q, K)] = popool.tile([P, 4 * IC], f16, tag="po16", name="po16")
            for half in range(2):
                po = po_ps.tile([P, IC], f32, tag="po")
                for c2 in range(2):
                    rt = rts[(2 * half + c2, q // 2)]
                    for rg in range(2):
                        c = 2 * c2 + rg
                        nc.tensor.matmul(
                            po[32 * c:32 * (c + 1), :],
                            w2s[64 * rg:64 * (rg + 1), :],
                            rt[64 * rg:64 * (rg + 1), IC * (q % 2):IC * (q % 2 + 1)],
                            start=True, stop=True,
                            tile_position=(64 * rg, 32 * c),
                        )
                dst16 = po16[(q, K)][:, 2 * IC * half + IC * ic:2 * IC * half + IC * (ic + 1)]
                if pcnt % 2 == 0:
                    nc.scalar.copy(dst16, po[:])
                else:
                    nc.vector.tensor_copy(dst16, po[:])
                pcnt += 1
        if ic == 1:
            for q in range(4):
                nc.gpsimd.dma_start(
                    out[q, K].rearrange("jo half ic i -> jo (half ic i)"),
                    po16[(q, K)][:],
                )

    blocks = [(K, ic) for K in range(NK) for ic in range(NIC)]
    prev = None
    for blk in blocks:
        rts = l1_block(*blk)
        if prev is not None:
            l2_block(prev[0][0], prev[0][1], prev[1])
        prev = (blk, rts)
    l2_block(prev[0][0], prev[0][1], prev[1])
    mlp.close()


def _build():
    key = "v5"
    if key in _CACHE:
        return _CACHE[key]
    from contextlib import ExitStack
    import concourse.tile as tile
    from concourse import bacc

    nc = bacc.Bacc()
    with tile.TileContext(nc) as tc:
        with ExitStack() as ctx:
            _emit(nc, tc, ctx)
    nc.compile()
    _CACHE[key] = nc
    return nc


def _install_ntff_shim():
    """Provide antenv.axon_hooks + ctypes NTFF hook so
    run_bass_kernel_spmd(trace=True) can profile."""
    import sys
    import types

    if "antenv.axon_hooks" in sys.modules:
        return
    mod = types.ModuleType("antenv.axon_hooks")
    mod._hook = None
    mod.set_axon_ntff_profile_hook = lambda h: setattr(mod, "_hook", h)
    mod.get_axon_ntff_profile_hook = lambda: mod._hook
    sys.modules["antenv.axon_hooks"] = mod
    try:
        from trn_agent_boot.trn_boot import _ntff_profile_via_ctypes

        mod._hook = _ntff_profile_via_ctypes("/opt/axon/libaxon_pjrt.so")
    except Exception as e:  # degrade to no-trace
        print(f"ntff shim install failed: {e}")


_EYE16 = None


def kernel(adj, mask, w1, b1, w2, b2):
    from concourse.bass_utils import run_bass_kernel_spmd

    global LAST_RESULTS, _EYE16
    adj = np.ascontiguousarray(np.asarray(adj, dtype=np.float32))
    mask = np.asarray(mask)
    w1 = np.asarray(w1, dtype=np.float32)
    b1 = np.asarray(b1, dtype=np.float32)
    w2 = np.asarray(w2, dtype=np.float32)
    b2 = np.asarray(b2, dtype=np.float32)
    assert adj.shape == (B, N, N), adj.shape

    m = mask.astype(np.float32)
    general_mask = not np.all(m == 1.0)
    if general_mask:
        pair = m[:, :, None] * m[:, None, :]
        adj = np.ascontiguousarray(adj * pair)

    trace = bool(int(os.environ.get("KERNEL_TRACE", "0")))
    if trace:
        _install_ntff_shim()
    nc = _build()

    if _EYE16 is None:
        _EYE16 = np.eye(N, dtype=np.float16)

    # w1big[32q + 8s + kk, 16jl + h] = delta(kk,jl) * w1[perm(s), h]
    perm = [1, 2, 3, 0]  # channels Q, Q2, Q4, eye -> stacks P1, P2, P4, self
    w1big = np.zeros((32, P), np.float32)
    for s in range(4):
        for k in range(8):
            w1big[8 * s + k, HID * k:HID * (k + 1)] = w1[perm[s]]
    w1big = np.tile(w1big, (4, 1)).astype(np.float16)
    # w2big[64r + 16jl + h, 8jl' + o] = delta(jl,jl') * w2[h, o]
    w2q = np.zeros((64, 32), np.float32)
    for jl in range(4):
        w2q[HID * jl:HID * (jl + 1), HEADS * jl:HEADS * (jl + 1)] = w2
    w2big = np.tile(w2q, (2, 1)).astype(np.float16)

    shared = {
        "eye16": _EYE16,
        "w1big": np.ascontiguousarray(w1big),
        "w2big": np.ascontiguousarray(w2big),
        "b1rep": np.ascontiguousarray(np.tile(b1, 8).astype(np.float32)[:, None]),
        "idn32": np.eye(P, dtype=np.float32),
    }
    in_maps = [{"adj": adj[c], **shared} for c in range(B)]
    res = run_bass_kernel_spmd(nc, in_maps, list(range(B)), trace=trace)
    LAST_RESULTS = res

    # out_raw[q, K, (16jl x 8o), half, ic, i'] -> out[i, j, o]
    # j = 256q + 32K + 16half + jl ; i = 512ic + i'
    outs = []
    for c in range(B):
        v = res.results[c]["out_raw"].reshape(4, NK, 16, HEADS, 2, NIC, IC)
        o = np.transpose(v, (5, 6, 0, 1, 4, 2, 3)).reshape(N, N, HEADS)
        outs.append(o.astype(np.float32))
    outp = np.stack(outs, axis=0)
    if np.any(b2 != 0.0):
        outp = outp + b2
    if general_mask:
        outp = outp * pair[..., None]
    return np.ascontiguousarray(outp.astype(np.float32))
